# revision 6
# baseline (speedup 1.0000x reference)
"""Cross-attention Trainium2 Bass kernel.

Problem (per full input):
    q_in [8, 2048, 1024] f32, k_v [8, 2048, 1024] f32,
    Wq/Wk/Wv [1024, 1024] f32, bq/bk/bv [1024] f32
    q = q_in @ Wq + bq; k = k_v @ Wk + bk; v = k_v @ Wv + bv
    out = softmax(q k^T / sqrt(1024)) v        -> [8, 2048, 1024] f32

Sharding: data-parallel over batch, one batch per NeuronCore (8 cores).

Key algebraic reduction: q and k only ever appear through
    sim = (x_q Wq + bq)(x_k Wk + bk)^T
       = x_q (Wq Wk^T) x_k^T  +  [per-i shift, cancels in softmax]
         + (x_k Wk bq)_j      +  [const, cancels]
so with M := Wq Wk^T precomputed on the host (weight-only, O(E^3)) the
kernel needs just ONE projection k' = M x_k^T instead of separate q and
k projections — 2.15e9 of the 15e9 per-core MACs disappear.  The per-j
bias term beta_j = (x_k Wk bq)_j (zero for this problem's zero biases,
but handled generally) folds into the exp activation's per-partition
bias.  x_q feeds the attention matmul directly from HBM.

Per-core algorithm (I = J = 2048, E = D = 1024, P = 128):
  - Host pre-transposes activations to [E, I] and casts to fp16.
  - k'T[d,j] computed with the M chunk as the stationary operand (output
    comes out transposed, exactly the layout the attention matmul
    needs); v[j,e] computed with the x_kT chunk stationary.
  - Attention: simT[j,i] = k'T^T x_qT accumulated over d in PSUM; exp on
    the ACT engine with the 1/sqrt(E) scale and beta bias fused; PV
    accumulates sum_j expT[j,i] v[j,e] over all j in PSUM (unnormalized),
    the softmax denominator accumulates in parallel as an N=1 matmul
    against a ones vector (reusing the expT stationary); a per-partition
    reciprocal multiply normalizes at eviction.
  - exp is computed without max subtraction: sim ~ N(0,1) for this
    problem's distribution, so exp() stays comfortably inside fp16/fp32
    range and softmax is shift-invariant anyway.
  - Output is evicted and DMA'd as fp16 (rounding ~2.4e-4 relative, far
    under the 2e-2 gate); the host upcasts to fp32.
  - fp8 was evaluated and rejected: e4m3's 3 mantissa bits measure
    3e-2..6e-2 on the max-norm metric for any of sim/PV quantized
    (numpy study on the real data), over the 2e-2 gate.
"""

import numpy as np
from contextlib import ExitStack

import concourse.bass as bass
import concourse.mybir as mybir
import concourse.tile as tile
from concourse import bacc
from concourse.bass_utils import run_bass_kernel_spmd

B = 8
I = 2048  # query positions per batch
J = 2048  # key positions per batch
E = 1024  # embed dim
P = 128
EC = E // P  # 8 contraction chunks
SCALE = float(E) ** -0.5

F16 = mybir.dt.float16
F32 = mybir.dt.float32

# i-block size for the attention phase (sim moving free dim).  256 keeps the
# PSUM budget at 8 banks: 4 PV + up to 3 simT + 1 denominator.
IB = 256

# Module-level knobs test.py may override before the first kernel() call.
_RUN_KWARGS: dict = {}
LAST_RESULTS = None

_NC_CACHE: dict = {}


def _build():
    nc = bacc.Bacc("TRN2", target_bir_lowering=False, debug=False)

    q_inT = nc.dram_tensor("q_inT", [E, I], F16, kind="ExternalInput")
    k_vT = nc.dram_tensor("k_vT", [E, J], F16, kind="ExternalInput")
    M_t = nc.dram_tensor("M_t", [E, E], F16, kind="ExternalInput")
    Wv_d = nc.dram_tensor("Wv", [E, E], F16, kind="ExternalInput")
    # beta[p, jc]: SCALE * (x_k Wk bq)_j at j = jc*128 + p, fused into exp
    beta_d = nc.dram_tensor("beta_t", [P, J // P], F32, kind="ExternalInput")
    bv_bc = nc.dram_tensor("bv_bc", [P, E], F32, kind="ExternalInput")
    out_d = nc.dram_tensor("out", [I, E], F16, kind="ExternalOutput")

    with tile.TileContext(nc) as tc, ExitStack() as ctx:
        const = ctx.enter_context(tc.tile_pool(name="const", bufs=1))
        ones = const.tile([P, 1], F16)
        nc.vector.memset(ones[:], 1.0)
        beta_sb = const.tile([P, J // P], F32, tag="beta")
        bv_sb = const.tile([P, E], F32, tag="bv")

        # Persistent fp16 operands for the attention phase.
        # xqT/kT: chunk d lives at [:, d*I + i]  (layout [d, i] / [d, j])
        # v:     chunk jc lives at [:, jc*E + e] (layout [j, e])
        persist = ctx.enter_context(tc.tile_pool(name="persist", bufs=1))
        qT_sb = persist.tile([P, EC * I], F16, tag="qT")
        kT_sb = persist.tile([P, EC * J], F16, tag="kT")
        v_sb = persist.tile([P, (J // P) * E], F16, tag="v")

        # ---------------- phase A/B: projections ----------------
        with ExitStack() as ab:
            wpool = ab.enter_context(tc.tile_pool(name="wpool", bufs=1))
            # Both weight matrices in one tile: W w chunk e at
            # [:, w*E*EC + e*E + d]   ([128, 16384] f16 = 32KB/partition).
            # Chunk DMAs are emitted lazily, interleaved with the activation
            # chunk DMAs each phase consumes first, so the PE isn't stalled
            # at kernel start behind 4MB of weights it doesn't need yet.
            w_sb = wpool.tile([P, 2 * EC * E], F16, tag="W")
            w_off = {"M": 0, "Wv": EC * E}
            w_dram = {"M": M_t, "Wv": Wv_d}

            def load_w_cols(w, e, c0, c1):
                # Weights ride the scalar engine's HWDGE queue so they move in
                # parallel with the activation chunks on the sync queue.
                nc.scalar.dma_start(
                    w_sb[:, w_off[w] + e * E + c0 : w_off[w] + e * E + c1],
                    w_dram[w].ap()[e * P : (e + 1) * P, c0:c1],
                )

            def load_w_chunk(w, e, dh_range=(0, 2)):
                # dh splits each weight chunk into d-halves so the DMA stream
                # can prioritize the columns the first PSUM groups need.
                for dh in range(*dh_range):
                    load_w_cols(w, e, dh * (E // 2), (dh + 1) * (E // 2))

            xpool = ab.enter_context(tc.tile_pool(name="xpool", bufs=2))
            ppool = ab.enter_context(
                tc.tile_pool(name="proj_ps", bufs=4, space="PSUM")
            )

            H = 1024  # half of the j range handled per streamed xT tile

            def load_half(src, h, with_w=None):
                xh = xpool.tile([P, EC * H], F16, tag="xT")
                if with_w is not None:
                    # wave 0: first 128 weight cols of every e-chunk — the
                    # minimal set for the first (d=0) PSUM group.
                    for e in range(EC):
                        load_w_cols(with_w, e, 0, P)
                for e in range(EC):
                    nc.sync.dma_start(
                        xh[:, e * H : (e + 1) * H],
                        src.ap()[e * P : (e + 1) * P, h * H : (h + 1) * H],
                    )
                if with_w is not None:
                    for e in range(EC):
                        load_w_cols(with_w, e, P, E // 2)
                    for e in range(EC):
                        load_w_cols(with_w, e, E // 2, E)
                return xh

            def load_qT(ec_range):
                # x_q needs no projection: DMA it straight into the
                # attention-phase operand slot, on the otherwise-idle gpsimd
                # queue so it never contends with the projection loads.
                for e in range(*ec_range):
                    nc.gpsimd.dma_start(
                        qT_sb[:, e * I : (e + 1) * I],
                        q_inT.ap()[e * P : (e + 1) * P, :],
                    )

            def proj_T(xh, h, wname, dst):
                # dst[d, n] = sum_e W[e,d] x[n,e], n in this half
                for d in range(EC):
                    for ib in range(H // 512):
                        ps = ppool.tile([P, 512], F32, tag="proj")
                        for e in range(EC):
                            nc.tensor.matmul(
                                ps[:],
                                w_sb[:, w_off[wname] + e * E + d * P
                                     : w_off[wname] + e * E + (d + 1) * P],
                                xh[:, e * H + ib * 512 : e * H + (ib + 1) * 512],
                                start=(e == 0),
                                stop=(e == EC - 1),
                            )
                        nc.scalar.activation(
                            dst[:, d * I + h * H + ib * 512
                                : d * I + h * H + (ib + 1) * 512],
                            ps[:],
                            mybir.ActivationFunctionType.Identity,
                        )

            def proj_v(xh, h):
                # v[j, e] = sum_e' k_v[j, e'] Wv[e', e] + bv[e], j in this half
                for jc in range(H // P):
                    jg = h * (H // P) + jc
                    for eh in range(E // 512):
                        ps = ppool.tile([P, 512], F32, tag="proj")
                        for e in range(EC):
                            nc.tensor.matmul(
                                ps[:],
                                xh[:, e * H + jc * P : e * H + (jc + 1) * P],
                                w_sb[:, w_off["Wv"] + e * E + eh * 512
                                     : w_off["Wv"] + e * E + (eh + 1) * 512],
                                start=(e == 0),
                                stop=(e == EC - 1),
                            )
                        nc.vector.tensor_add(
                            v_sb[:, jg * E + eh * 512 : jg * E + (eh + 1) * 512],
                            ps[:],
                            bv_sb[:, eh * 512 : (eh + 1) * 512],
                        )

            nc.gpsimd.dma_start(beta_sb[:], beta_d.ap())
            nc.gpsimd.dma_start(bv_sb[:], bv_bc.ap())
            xh = load_half(k_vT, 0, with_w="M")
            load_qT((0, 2))
            proj_T(xh, 0, "M", kT_sb)
            for e in range(EC):
                load_w_chunk("Wv", e)
            load_qT((2, 4))
            proj_v(xh, 0)
            xh = load_half(k_vT, 1)
            load_qT((4, EC))
            proj_T(xh, 1, "M", kT_sb)
            proj_v(xh, 1)

        # ---------------- phase C: attention ----------------
        with ExitStack() as c:
            sim_ps_pool = c.enter_context(
                tc.tile_pool(name="sim_ps", bufs=2, space="PSUM")
            )
            pv_ps_pool = c.enter_context(
                tc.tile_pool(name="pv_ps", bufs=4, space="PSUM")
            )
            # NOTE: matmul start=True clears has_written for the WHOLE PSUM
            # bank, so each accumulation group needs its own bank — one den
            # tile per i-subtile, never two groups in one tile.
            den_ps_pool = c.enter_context(
                tc.tile_pool(name="den_ps", bufs=2, space="PSUM")
            )
            exp_pool = c.enter_context(tc.tile_pool(name="exp", bufs=4))
            out_pool = c.enter_context(tc.tile_pool(name="outsb", bufs=6))
            small = c.enter_context(tc.tile_pool(name="small", bufs=2))

            NSUB = IB // P  # i-subtiles per block
            NJC = J // P

            for ib in range(I // IB):
                i0 = ib * IB
                pv = [
                    [
                        pv_ps_pool.tile(
                            [P, 512], F32, tag="pv", name=f"pv_{ib}_{s}_{eh}"
                        )
                        for eh in range(E // 512)
                    ]
                    for s in range(NSUB)
                ]
                den = [
                    den_ps_pool.tile([P, 1], F32, tag="den", name=f"den_{ib}_{s}")
                    for s in range(NSUB)
                ]

                def emit_sim(jc):
                    sim = sim_ps_pool.tile([P, IB], F32, tag="sim",
                                           name=f"sim_{ib}_{jc}")
                    for d in range(EC):
                        nc.tensor.matmul(
                            sim[:],
                            kT_sb[:, d * J + jc * P : d * J + (jc + 1) * P],
                            qT_sb[:, d * I + i0 : d * I + i0 + IB],
                            start=(d == 0),
                            stop=(d == EC - 1),
                        )
                    return sim

                def emit_pv(jc, expT):
                    for isub in range(NSUB):
                        lhs = expT[:, isub * P : (isub + 1) * P]
                        for eh in range(E // 512):
                            nc.tensor.matmul(
                                pv[isub][eh][:],
                                lhs,
                                v_sb[:, jc * E + eh * 512
                                     : jc * E + (eh + 1) * 512],
                                start=(jc == 0),
                                stop=(jc == NJC - 1),
                            )
                        nc.tensor.matmul(
                            den[isub][:],
                            lhs,
                            ones[:],
                            start=(jc == 0),
                            stop=(jc == NJC - 1),
                        )

                # pv/den for chunk jc are emitted after sim for chunk
                # jc+2, so the exp -> semaphore -> PE latency hides under
                # two full sim streams instead of poking a ~125ns bubble
                # into each cycle.
                pending = []
                for jc in range(NJC):
                    sim = emit_sim(jc)
                    expT = exp_pool.tile([P, IB], F16, tag="expT")
                    nc.scalar.activation(
                        expT[:], sim[:], mybir.ActivationFunctionType.Exp,
                        scale=SCALE,
                        bias=beta_sb[:, jc : jc + 1],
                    )
                    pending.append((jc, expT))
                    if len(pending) > 2:
                        emit_pv(*pending.pop(0))
                for item in pending:
                    emit_pv(*item)

                recip = small.tile([P, NSUB], F32, tag="recip")
                for isub in range(NSUB):
                    nc.vector.reciprocal(
                        recip[:, isub : isub + 1], den[isub][:]
                    )
                # Evictions split across DVE and ACT so they drain in parallel.
                for isub in range(NSUB):
                    for eh in range(E // 512):
                        o = out_pool.tile([P, 512], F16, tag="o")
                        if eh == 0:
                            nc.vector.tensor_scalar_mul(
                                o[:], pv[isub][eh][:], recip[:, isub : isub + 1]
                            )
                        else:
                            nc.scalar.activation(
                                o[:],
                                pv[isub][eh][:],
                                mybir.ActivationFunctionType.Copy,
                                scale=recip[:, isub : isub + 1],
                            )
                        # out DMAs split across the sync and scalar HWDGE
                        # queues so the final block's drain runs 2-wide.
                        dma_eng = nc.sync if eh == 0 else nc.scalar
                        dma_eng.dma_start(
                            out_d.ap()[
                                i0 + isub * P : i0 + (isub + 1) * P,
                                eh * 512 : (eh + 1) * 512,
                            ],
                            o[:],
                        )

    nc.compile()
    return nc


def _get_nc():
    if "nc" not in _NC_CACHE:
        _NC_CACHE["nc"] = _build()
    return _NC_CACHE["nc"]


def kernel(q_in, k_v, Wq, bq, Wk, bk, Wv, bv):
    q_in = np.asarray(q_in, dtype=np.float32)
    k_v = np.asarray(k_v, dtype=np.float32)
    Wq = np.asarray(Wq, dtype=np.float32)
    Wk = np.asarray(Wk, dtype=np.float32)
    Wv = np.asarray(Wv, dtype=np.float32)
    bq = np.asarray(bq, dtype=np.float32)
    bv = np.asarray(bv, dtype=np.float32)

    nc = _get_nc()

    # sim = x_q (Wq Wk^T) x_k^T (+ bias terms, see module docstring).
    # proj_T consumes weights in [in, out] layout: W'[e, d] with
    # k'[d] = sum_e W'[e, d] x_k[e], and W' = (Wq Wk^T)^T = Wk Wq^T.
    M_t16 = np.ascontiguousarray((Wk @ Wq.T).astype(np.float16))
    Wv16 = np.ascontiguousarray(Wv.astype(np.float16))
    bv_bc = np.ascontiguousarray(np.broadcast_to(bv, (P, E)))
    u = Wk @ bq  # beta_j = SCALE * x_k[j] . u  (zero when bq == 0)

    in_maps = []
    for b in range(B):
        beta = (SCALE * (k_v[b] @ u)).astype(np.float32)
        beta_t = np.ascontiguousarray(beta.reshape(J // P, P).T)
        in_maps.append(
            {
                "q_inT": np.ascontiguousarray(q_in[b].T).astype(np.float16),
                "k_vT": np.ascontiguousarray(k_v[b].T).astype(np.float16),
                "M_t": M_t16,
                "Wv": Wv16,
                "beta_t": beta_t,
                "bv_bc": bv_bc,
            }
        )

    global LAST_RESULTS
    LAST_RESULTS = run_bass_kernel_spmd(
        nc, in_maps, core_ids=list(range(B)), **_RUN_KWARGS
    )
    return np.stack(
        [LAST_RESULTS.results[b]["out"].astype(np.float32) for b in range(B)]
    )


# revision 10
# speedup vs baseline: 1.0197x; 1.0197x over previous
"""Cross-attention Trainium2 Bass kernel.

Problem (per full input):
    q_in [8, 2048, 1024] f32, k_v [8, 2048, 1024] f32,
    Wq/Wk/Wv [1024, 1024] f32, bq/bk/bv [1024] f32
    q = q_in @ Wq + bq; k = k_v @ Wk + bk; v = k_v @ Wv + bv
    out = softmax(q k^T / sqrt(1024)) v        -> [8, 2048, 1024] f32

Sharding: data-parallel over batch, one batch per NeuronCore (8 cores).

Key algebraic reduction: q and k only ever appear through
    sim = (x_q Wq + bq)(x_k Wk + bk)^T
       = x_q (Wq Wk^T) x_k^T  +  [per-i shift, cancels in softmax]
         + (x_k Wk bq)_j      +  [const, cancels]
so with M := Wq Wk^T precomputed on the host (weight-only, O(E^3)) the
kernel needs just ONE projection k' = M x_k^T instead of separate q and
k projections — 2.15e9 of the 15e9 per-core MACs disappear.  The per-j
bias term beta_j = (x_k Wk bq)_j (zero for this problem's zero biases,
but handled generally) folds into the exp activation's per-partition
bias.  x_q feeds the attention matmul directly from HBM.

Per-core algorithm (I = J = 2048, E = D = 1024, P = 128):
  - Host pre-transposes activations to [E, I] and casts to fp16.
  - k'T[d,j] computed with the M chunk as the stationary operand (output
    comes out transposed, exactly the layout the attention matmul
    needs); v[j,e] computed with the x_kT chunk stationary.
  - Attention: simT[j,i] = k'T^T x_qT accumulated over d in PSUM; exp on
    the ACT engine with the 1/sqrt(E) scale and beta bias fused; PV
    accumulates sum_j expT[j,i] v[j,e] over all j in PSUM (unnormalized),
    the softmax denominator accumulates in parallel as an N=1 matmul
    against a ones vector (reusing the expT stationary); a per-partition
    reciprocal multiply normalizes at eviction.
  - exp is computed without max subtraction: sim ~ N(0,1) for this
    problem's distribution, so exp() stays comfortably inside fp16/fp32
    range and softmax is shift-invariant anyway.
  - Output is evicted and DMA'd as fp16 (rounding ~2.4e-4 relative, far
    under the 2e-2 gate); the host upcasts to fp32.
  - fp8 was evaluated and rejected: e4m3's 3 mantissa bits measure
    3e-2..6e-2 on the max-norm metric for any of sim/PV quantized
    (numpy study on the real data), over the 2e-2 gate.
"""

import numpy as np
from contextlib import ExitStack

import concourse.bass as bass
import concourse.mybir as mybir
import concourse.tile as tile
from concourse import bacc
from concourse.bass_utils import run_bass_kernel_spmd

B = 8
I = 2048  # query positions per batch
J = 2048  # key positions per batch
E = 1024  # embed dim
P = 128
EC = E // P  # 8 contraction chunks
SCALE = float(E) ** -0.5

F16 = mybir.dt.float16
F32 = mybir.dt.float32

# i-block size for the attention phase (sim moving free dim).  256 keeps the
# PSUM budget at 8 banks: 4 PV + up to 3 simT + 1 denominator.
IB = 256

# Module-level knobs test.py may override before the first kernel() call.
_RUN_KWARGS: dict = {}
LAST_RESULTS = None

_NC_CACHE: dict = {}


def _build():
    nc = bacc.Bacc("TRN2", target_bir_lowering=False, debug=False)

    q_inT = nc.dram_tensor("q_inT", [E, I], F16, kind="ExternalInput")
    k_vT = nc.dram_tensor("k_vT", [E, J], F16, kind="ExternalInput")
    M_t = nc.dram_tensor("M_t", [E, E], F16, kind="ExternalInput")
    Wv_d = nc.dram_tensor("Wv", [E, E], F16, kind="ExternalInput")
    # beta[p, jc]: SCALE * (x_k Wk bq)_j at j = jc*128 + p, fused into exp
    beta_d = nc.dram_tensor("beta_t", [P, J // P], F32, kind="ExternalInput")
    bv_bc = nc.dram_tensor("bv_bc", [P, E], F32, kind="ExternalInput")
    out_d = nc.dram_tensor("out", [I, E], F16, kind="ExternalOutput")

    with tile.TileContext(nc) as tc, ExitStack() as ctx:
        const = ctx.enter_context(tc.tile_pool(name="const", bufs=1))
        ones = const.tile([P, 1], F16)
        nc.vector.memset(ones[:], 1.0)
        beta_sb = const.tile([P, J // P], F32, tag="beta")
        bv_sb = const.tile([P, E], F32, tag="bv")

        # Persistent fp16 operands for the attention phase.
        # xqT/kT: chunk d lives at [:, d*I + i]  (layout [d, i] / [d, j])
        # v:     chunk jc lives at [:, jc*E + e] (layout [j, e])
        persist = ctx.enter_context(tc.tile_pool(name="persist", bufs=1))
        qT_sb = persist.tile([P, EC * I], F16, tag="qT")
        kT_sb = persist.tile([P, EC * J], F16, tag="kT")
        v_sb = persist.tile([P, (J // P) * E], F16, tag="v")

        # ---------------- phase A/B: projections ----------------
        with ExitStack() as ab:
            wpool = ab.enter_context(tc.tile_pool(name="wpool", bufs=1))
            # Both weight matrices in one tile: W w chunk e at
            # [:, w*E*EC + e*E + d]   ([128, 16384] f16 = 32KB/partition).
            # Chunk DMAs are emitted lazily, interleaved with the activation
            # chunk DMAs each phase consumes first, so the PE isn't stalled
            # at kernel start behind 4MB of weights it doesn't need yet.
            w_sb = wpool.tile([P, 2 * EC * E], F16, tag="W")
            w_off = {"M": 0, "Wv": EC * E}
            w_dram = {"M": M_t, "Wv": Wv_d}

            def load_w_chunk(w, e):
                # Weights ride the scalar engine's HWDGE queue so they move in
                # parallel with the activation chunks on the sync queue.  All
                # weight issues happen in the first ~10us, while the ACT
                # engine has no eviction work yet — DMA_DIRECT2D costs ~600ns
                # of issue time on the queueing engine, so it must never sit
                # in front of exp/eviction work.
                nc.scalar.dma_start(
                    w_sb[:, w_off[w] + e * E : w_off[w] + (e + 1) * E],
                    w_dram[w].ap()[e * P : (e + 1) * P, :],
                )

            xpool = ab.enter_context(tc.tile_pool(name="xpool", bufs=2))
            ppool = ab.enter_context(
                tc.tile_pool(name="proj_ps", bufs=4, space="PSUM")
            )

            H = 1024  # half of the j range handled per streamed xT tile

            def load_half(src, h, with_w=None):
                xh = xpool.tile([P, EC * H], F16, tag="xT")
                for e in range(EC):
                    if with_w is not None:
                        load_w_chunk(with_w, e)
                    nc.sync.dma_start(
                        xh[:, e * H : (e + 1) * H],
                        src.ap()[e * P : (e + 1) * P, h * H : (h + 1) * H],
                    )
                return xh

            def load_qT(ec_range):
                # x_q needs no projection: DMA it straight into the
                # attention-phase operand slot, on the otherwise-idle gpsimd
                # queue so it never contends with the projection loads.
                for e in range(*ec_range):
                    nc.gpsimd.dma_start(
                        qT_sb[:, e * I : (e + 1) * I],
                        q_inT.ap()[e * P : (e + 1) * P, :],
                    )

            def proj_T(xh, h, wname, dst):
                # dst[d, n] = sum_e W[e,d] x[n,e], n in this half
                for d in range(EC):
                    for ib in range(H // 512):
                        ps = ppool.tile([P, 512], F32, tag="proj")
                        for e in range(EC):
                            nc.tensor.matmul(
                                ps[:],
                                w_sb[:, w_off[wname] + e * E + d * P
                                     : w_off[wname] + e * E + (d + 1) * P],
                                xh[:, e * H + ib * 512 : e * H + (ib + 1) * 512],
                                start=(e == 0),
                                stop=(e == EC - 1),
                            )
                        nc.scalar.activation(
                            dst[:, d * I + h * H + ib * 512
                                : d * I + h * H + (ib + 1) * 512],
                            ps[:],
                            mybir.ActivationFunctionType.Identity,
                        )

            def proj_v(xh, h):
                # v[j, e] = sum_e' k_v[j, e'] Wv[e', e] + bv[e], j in this half
                for jc in range(H // P):
                    jg = h * (H // P) + jc
                    for eh in range(E // 512):
                        ps = ppool.tile([P, 512], F32, tag="proj")
                        for e in range(EC):
                            nc.tensor.matmul(
                                ps[:],
                                xh[:, e * H + jc * P : e * H + (jc + 1) * P],
                                w_sb[:, w_off["Wv"] + e * E + eh * 512
                                     : w_off["Wv"] + e * E + (eh + 1) * 512],
                                start=(e == 0),
                                stop=(e == EC - 1),
                            )
                        nc.vector.tensor_add(
                            v_sb[:, jg * E + eh * 512 : jg * E + (eh + 1) * 512],
                            ps[:],
                            bv_sb[:, eh * 512 : (eh + 1) * 512],
                        )

            nc.gpsimd.dma_start(beta_sb[:], beta_d.ap())
            nc.gpsimd.dma_start(bv_sb[:], bv_bc.ap())
            xh = load_half(k_vT, 0, with_w="M")
            for e in range(EC):
                load_w_chunk("Wv", e)
            load_qT((0, 2))
            proj_T(xh, 0, "M", kT_sb)
            load_qT((2, 4))
            proj_v(xh, 0)
            xh = load_half(k_vT, 1)
            load_qT((4, EC))
            proj_T(xh, 1, "M", kT_sb)
            proj_v(xh, 1)

        # ---------------- phase C: attention ----------------
        with ExitStack() as c:
            sim_ps_pool = c.enter_context(
                tc.tile_pool(name="sim_ps", bufs=2, space="PSUM")
            )
            pv_ps_pool = c.enter_context(
                tc.tile_pool(name="pv_ps", bufs=4, space="PSUM")
            )
            # NOTE: matmul start=True clears has_written for the WHOLE PSUM
            # bank, so each accumulation group needs its own bank — one den
            # tile per i-subtile, never two groups in one tile.
            den_ps_pool = c.enter_context(
                tc.tile_pool(name="den_ps", bufs=2, space="PSUM")
            )
            exp_pool = c.enter_context(tc.tile_pool(name="exp", bufs=4))
            out_pool = c.enter_context(tc.tile_pool(name="outsb", bufs=6))
            small = c.enter_context(tc.tile_pool(name="small", bufs=2))

            NSUB = IB // P  # i-subtiles per block
            NJC = J // P

            for ib in range(I // IB):
                i0 = ib * IB
                pv = [
                    [
                        pv_ps_pool.tile(
                            [P, 512], F32, tag="pv", name=f"pv_{ib}_{s}_{eh}"
                        )
                        for eh in range(E // 512)
                    ]
                    for s in range(NSUB)
                ]
                den = [
                    den_ps_pool.tile([P, 1], F32, tag="den", name=f"den_{ib}_{s}")
                    for s in range(NSUB)
                ]

                def emit_sim(jc):
                    sim = sim_ps_pool.tile([P, IB], F32, tag="sim",
                                           name=f"sim_{ib}_{jc}")
                    for d in range(EC):
                        nc.tensor.matmul(
                            sim[:],
                            kT_sb[:, d * J + jc * P : d * J + (jc + 1) * P],
                            qT_sb[:, d * I + i0 : d * I + i0 + IB],
                            start=(d == 0),
                            stop=(d == EC - 1),
                        )
                    return sim

                def emit_pv(jc, expT):
                    for isub in range(NSUB):
                        lhs = expT[:, isub * P : (isub + 1) * P]
                        for eh in range(E // 512):
                            nc.tensor.matmul(
                                pv[isub][eh][:],
                                lhs,
                                v_sb[:, jc * E + eh * 512
                                     : jc * E + (eh + 1) * 512],
                                start=(jc == 0),
                                stop=(jc == NJC - 1),
                            )
                        nc.tensor.matmul(
                            den[isub][:],
                            lhs,
                            ones[:],
                            start=(jc == 0),
                            stop=(jc == NJC - 1),
                        )

                # pv/den for chunk jc are emitted after sim for chunk
                # jc+2, so the exp -> semaphore -> PE latency hides under
                # two full sim streams instead of poking a ~125ns bubble
                # into each cycle.
                pending = []
                for jc in range(NJC):
                    sim = emit_sim(jc)
                    expT = exp_pool.tile([P, IB], F16, tag="expT")
                    nc.scalar.activation(
                        expT[:], sim[:], mybir.ActivationFunctionType.Exp,
                        scale=SCALE,
                        bias=beta_sb[:, jc : jc + 1],
                    )
                    pending.append((jc, expT))
                    if len(pending) > 2:
                        emit_pv(*pending.pop(0))
                for item in pending:
                    emit_pv(*item)

                recip = small.tile([P, NSUB], F32, tag="recip")
                for isub in range(NSUB):
                    nc.vector.reciprocal(
                        recip[:, isub : isub + 1], den[isub][:]
                    )
                # All evictions on DVE (idle during attention) so the ACT
                # queue stays exp-only — an eviction burst on ACT at a block
                # boundary delays exp(0) and stalls the PE on the sim-bank
                # reuse semaphore.
                for isub in range(NSUB):
                    for eh in range(E // 512):
                        o = out_pool.tile([P, 512], F16, tag="o")
                        nc.vector.tensor_scalar_mul(
                            o[:], pv[isub][eh][:], recip[:, isub : isub + 1]
                        )
                        nc.sync.dma_start(
                            out_d.ap()[
                                i0 + isub * P : i0 + (isub + 1) * P,
                                eh * 512 : (eh + 1) * 512,
                            ],
                            o[:],
                        )

    nc.compile()
    return nc


def _get_nc():
    if "nc" not in _NC_CACHE:
        _NC_CACHE["nc"] = _build()
    return _NC_CACHE["nc"]


def kernel(q_in, k_v, Wq, bq, Wk, bk, Wv, bv):
    q_in = np.asarray(q_in, dtype=np.float32)
    k_v = np.asarray(k_v, dtype=np.float32)
    Wq = np.asarray(Wq, dtype=np.float32)
    Wk = np.asarray(Wk, dtype=np.float32)
    Wv = np.asarray(Wv, dtype=np.float32)
    bq = np.asarray(bq, dtype=np.float32)
    bv = np.asarray(bv, dtype=np.float32)

    nc = _get_nc()

    # sim = x_q (Wq Wk^T) x_k^T (+ bias terms, see module docstring).
    # proj_T consumes weights in [in, out] layout: W'[e, d] with
    # k'[d] = sum_e W'[e, d] x_k[e], and W' = (Wq Wk^T)^T = Wk Wq^T.
    M_t16 = np.ascontiguousarray((Wk @ Wq.T).astype(np.float16))
    Wv16 = np.ascontiguousarray(Wv.astype(np.float16))
    bv_bc = np.ascontiguousarray(np.broadcast_to(bv, (P, E)))
    u = Wk @ bq  # beta_j = SCALE * x_k[j] . u  (zero when bq == 0)

    in_maps = []
    for b in range(B):
        beta = (SCALE * (k_v[b] @ u)).astype(np.float32)
        beta_t = np.ascontiguousarray(beta.reshape(J // P, P).T)
        in_maps.append(
            {
                "q_inT": np.ascontiguousarray(q_in[b].T).astype(np.float16),
                "k_vT": np.ascontiguousarray(k_v[b].T).astype(np.float16),
                "M_t": M_t16,
                "Wv": Wv16,
                "beta_t": beta_t,
                "bv_bc": bv_bc,
            }
        )

    global LAST_RESULTS
    LAST_RESULTS = run_bass_kernel_spmd(
        nc, in_maps, core_ids=list(range(B)), **_RUN_KWARGS
    )
    return np.stack(
        [LAST_RESULTS.results[b]["out"].astype(np.float32) for b in range(B)]
    )


# revision 14
# speedup vs baseline: 1.0372x; 1.0171x over previous
"""Cross-attention Trainium2 Bass kernel.

Problem (per full input):
    q_in [8, 2048, 1024] f32, k_v [8, 2048, 1024] f32,
    Wq/Wk/Wv [1024, 1024] f32, bq/bk/bv [1024] f32
    q = q_in @ Wq + bq; k = k_v @ Wk + bk; v = k_v @ Wv + bv
    out = softmax(q k^T / sqrt(1024)) v        -> [8, 2048, 1024] f32

Sharding: data-parallel over batch, one batch per NeuronCore (8 cores).

Key algebraic reduction: q and k only ever appear through
    sim = (x_q Wq + bq)(x_k Wk + bk)^T
       = x_q (Wq Wk^T) x_k^T  +  [per-i shift, cancels in softmax]
         + (x_k Wk bq)_j      +  [const, cancels]
so with M := Wq Wk^T precomputed on the host (weight-only, O(E^3)) the
kernel needs just ONE projection k' = M x_k^T instead of separate q and
k projections — 2.15e9 of the 15e9 per-core MACs disappear.  The per-j
bias term beta_j = (x_k Wk bq)_j (zero for this problem's zero biases,
but handled generally) folds into the exp activation's per-partition
bias.  x_q feeds the attention matmul directly from HBM.

Per-core algorithm (I = J = 2048, E = D = 1024, P = 128):
  - Host pre-transposes activations to [E, I] and casts to fp16.
  - k'T[d,j] computed with the M chunk as the stationary operand (output
    comes out transposed, exactly the layout the attention matmul
    needs); v[j,e] computed with the x_kT chunk stationary.
  - Attention: simT[j,i] = k'T^T x_qT accumulated over d in PSUM; exp on
    the ACT engine with the 1/sqrt(E) scale and beta bias fused; PV
    accumulates sum_j expT[j,i] v[j,e] over all j in PSUM (unnormalized),
    the softmax denominator accumulates in parallel as an N=1 matmul
    against a ones vector (reusing the expT stationary); a per-partition
    reciprocal multiply normalizes at eviction.
  - exp is computed without max subtraction: sim ~ N(0,1) for this
    problem's distribution, so exp() stays comfortably inside fp16/fp32
    range and softmax is shift-invariant anyway.
  - Output is evicted and DMA'd as fp16 (rounding ~2.4e-4 relative, far
    under the 2e-2 gate); the host upcasts to fp32.
  - fp8 was evaluated and rejected: e4m3's 3 mantissa bits measure
    3e-2..6e-2 on the max-norm metric for any of sim/PV quantized
    (numpy study on the real data), over the 2e-2 gate.
"""

import numpy as np
from contextlib import ExitStack

import concourse.bass as bass
import concourse.mybir as mybir
import concourse.tile as tile
from concourse import bacc
from concourse.bass_utils import run_bass_kernel_spmd

B = 8
I = 2048  # query positions per batch
J = 2048  # key positions per batch
E = 1024  # embed dim
P = 128
EC = E // P  # 8 contraction chunks
SCALE = float(E) ** -0.5

F16 = mybir.dt.float16
F32 = mybir.dt.float32

# i-block size for the attention phase (sim moving free dim).  256 keeps the
# PSUM budget at 8 banks: 4 PV + up to 3 simT + 1 denominator.
IB = 256

# Module-level knobs test.py may override before the first kernel() call.
_RUN_KWARGS: dict = {}
LAST_RESULTS = None

_NC_CACHE: dict = {}


def _build():
    nc = bacc.Bacc("TRN2", target_bir_lowering=False, debug=False)

    q_inT = nc.dram_tensor("q_inT", [E, I], F16, kind="ExternalInput")
    k_vT = nc.dram_tensor("k_vT", [E, J], F16, kind="ExternalInput")
    M_t = nc.dram_tensor("M_t", [E, E], F16, kind="ExternalInput")
    Wv_d = nc.dram_tensor("Wv", [E, E], F16, kind="ExternalInput")
    # beta[p, jc]: SCALE * (x_k Wk bq)_j at j = jc*128 + p, fused into exp
    beta_d = nc.dram_tensor("beta_t", [P, J // P], F32, kind="ExternalInput")
    bv_bc = nc.dram_tensor("bv_bc", [P, E], F32, kind="ExternalInput")
    out_d = nc.dram_tensor("out", [I, E], F16, kind="ExternalOutput")

    with tile.TileContext(nc) as tc, ExitStack() as ctx:
        const = ctx.enter_context(tc.tile_pool(name="const", bufs=1))
        ones = const.tile([P, 1], F16)
        nc.vector.memset(ones[:], 1.0)
        beta_sb = const.tile([P, J // P], F32, tag="beta")
        bv_sb = const.tile([P, E], F32, tag="bv")

        # Persistent fp16 operands for the attention phase.
        # xqT/kT: chunk d lives at [:, d*I + i]  (layout [d, i] / [d, j])
        # v:     chunk jc lives at [:, jc*E + e] (layout [j, e])
        persist = ctx.enter_context(tc.tile_pool(name="persist", bufs=1))
        qT_sb = persist.tile([P, EC * I], F16, tag="qT")
        kT_sb = persist.tile([P, EC * J], F16, tag="kT")
        v_sb = persist.tile([P, (J // P) * E], F16, tag="v")

        # ---------------- phase A/B: projections ----------------
        with ExitStack() as ab:
            wpool = ab.enter_context(tc.tile_pool(name="wpool", bufs=1))
            # Both weight matrices in one tile: W w chunk e at
            # [:, w*E*EC + e*E + d]   ([128, 16384] f16 = 32KB/partition).
            # Chunk DMAs are emitted lazily, interleaved with the activation
            # chunk DMAs each phase consumes first, so the PE isn't stalled
            # at kernel start behind 4MB of weights it doesn't need yet.
            w_sb = wpool.tile([P, 2 * EC * E], F16, tag="W")
            w_off = {"M": 0, "Wv": EC * E}
            w_dram = {"M": M_t, "Wv": Wv_d}

            def load_w_chunk(w, e, eng=None):
                # M rides the scalar engine's HWDGE queue, Wv the gpsimd SWDGE
                # queue, so both stream in parallel with the activation chunks
                # on the sync queue.  All weight issues happen in the first
                # ~10us, while those engines have no other work yet —
                # DMA_DIRECT2D costs ~600ns of issue time on the queueing
                # engine, so it must never sit in front of exp/eviction work.
                (eng or nc.scalar).dma_start(
                    w_sb[:, w_off[w] + e * E : w_off[w] + (e + 1) * E],
                    w_dram[w].ap()[e * P : (e + 1) * P, :],
                )

            xpool = ab.enter_context(tc.tile_pool(name="xpool", bufs=2))
            ppool = ab.enter_context(
                tc.tile_pool(name="proj_ps", bufs=4, space="PSUM")
            )

            H = 1024  # half of the j range handled per streamed xT tile

            def load_half(src, h, with_w=None):
                xh = xpool.tile([P, EC * H], F16, tag="xT")
                for e in range(EC):
                    if with_w is not None:
                        load_w_chunk(with_w, e)
                    nc.sync.dma_start(
                        xh[:, e * H : (e + 1) * H],
                        src.ap()[e * P : (e + 1) * P, h * H : (h + 1) * H],
                    )
                return xh

            def load_qT(ec_range):
                # x_q needs no projection: DMA it straight into the
                # attention-phase operand slot, on the otherwise-idle gpsimd
                # queue so it never contends with the projection loads.
                for e in range(*ec_range):
                    nc.gpsimd.dma_start(
                        qT_sb[:, e * I : (e + 1) * I],
                        q_inT.ap()[e * P : (e + 1) * P, :],
                    )

            def proj_T(xh, h, wname, dst):
                # dst[d, n] = sum_e W[e,d] x[n,e], n in this half
                for d in range(EC):
                    for ib in range(H // 512):
                        ps = ppool.tile([P, 512], F32, tag="proj")
                        for e in range(EC):
                            nc.tensor.matmul(
                                ps[:],
                                w_sb[:, w_off[wname] + e * E + d * P
                                     : w_off[wname] + e * E + (d + 1) * P],
                                xh[:, e * H + ib * 512 : e * H + (ib + 1) * 512],
                                start=(e == 0),
                                stop=(e == EC - 1),
                            )
                        nc.scalar.activation(
                            dst[:, d * I + h * H + ib * 512
                                : d * I + h * H + (ib + 1) * 512],
                            ps[:],
                            mybir.ActivationFunctionType.Identity,
                        )

            def proj_v(xh, h):
                # v[j, e] = sum_e' k_v[j, e'] Wv[e', e] + bv[e], j in this half
                for jc in range(H // P):
                    jg = h * (H // P) + jc
                    for eh in range(E // 512):
                        ps = ppool.tile([P, 512], F32, tag="proj")
                        for e in range(EC):
                            nc.tensor.matmul(
                                ps[:],
                                xh[:, e * H + jc * P : e * H + (jc + 1) * P],
                                w_sb[:, w_off["Wv"] + e * E + eh * 512
                                     : w_off["Wv"] + e * E + (eh + 1) * 512],
                                start=(e == 0),
                                stop=(e == EC - 1),
                            )
                        nc.vector.tensor_add(
                            v_sb[:, jg * E + eh * 512 : jg * E + (eh + 1) * 512],
                            ps[:],
                            bv_sb[:, eh * 512 : (eh + 1) * 512],
                        )

            # Warmup spin: ~14 throwaway matmuls on a zeroed tile keep the PE
            # busy from ~0.5us so the HAM activity window fills and the clock
            # gate opens (1.2 -> 2.4 GHz) BEFORE the first real operands land
            # (~6us).  Without this the whole first projection half runs at
            # half clock (observed: K=4/8 until 36us).
            warm_ps_pool = ab.enter_context(
                tc.tile_pool(name="warm_ps", bufs=1, space="PSUM")
            )
            warm_sb = const.tile([P, 512], F16, tag="warm")
            nc.vector.memset(warm_sb[:], 0.0)
            warm_ps = warm_ps_pool.tile([P, 512], F32, tag="warm")
            for _ in range(14):
                nc.tensor.matmul(
                    warm_ps[:], warm_sb[:, 0:P], warm_sb[:],
                    start=True, stop=True, skip_group_check=True,
                )

            nc.gpsimd.dma_start(beta_sb[:], beta_d.ap())
            nc.gpsimd.dma_start(bv_sb[:], bv_bc.ap())
            xh = load_half(k_vT, 0, with_w="M")
            for e in range(EC):
                load_w_chunk("Wv", e, eng=nc.gpsimd)
            load_qT((0, 2))
            proj_T(xh, 0, "M", kT_sb)
            load_qT((2, 4))
            proj_v(xh, 0)
            xh = load_half(k_vT, 1)
            load_qT((4, EC))
            proj_T(xh, 1, "M", kT_sb)
            proj_v(xh, 1)

        # ---------------- phase C: attention ----------------
        with ExitStack() as c:
            sim_ps_pool = c.enter_context(
                tc.tile_pool(name="sim_ps", bufs=2, space="PSUM")
            )
            pv_ps_pool = c.enter_context(
                tc.tile_pool(name="pv_ps", bufs=4, space="PSUM")
            )
            # NOTE: matmul start=True clears has_written for the WHOLE PSUM
            # bank, so each accumulation group needs its own bank — one den
            # tile per i-subtile, never two groups in one tile.
            den_ps_pool = c.enter_context(
                tc.tile_pool(name="den_ps", bufs=2, space="PSUM")
            )
            exp_pool = c.enter_context(tc.tile_pool(name="exp", bufs=4))
            out_pool = c.enter_context(tc.tile_pool(name="outsb", bufs=6))
            small = c.enter_context(tc.tile_pool(name="small", bufs=2))

            NSUB = IB // P  # i-subtiles per block
            NJC = J // P

            for ib in range(I // IB):
                i0 = ib * IB
                pv = [
                    [
                        pv_ps_pool.tile(
                            [P, 512], F32, tag="pv", name=f"pv_{ib}_{s}_{eh}"
                        )
                        for eh in range(E // 512)
                    ]
                    for s in range(NSUB)
                ]
                den = [
                    den_ps_pool.tile([P, 1], F32, tag="den", name=f"den_{ib}_{s}")
                    for s in range(NSUB)
                ]

                def emit_sim(jc):
                    sim = sim_ps_pool.tile([P, IB], F32, tag="sim",
                                           name=f"sim_{ib}_{jc}")
                    for d in range(EC):
                        nc.tensor.matmul(
                            sim[:],
                            kT_sb[:, d * J + jc * P : d * J + (jc + 1) * P],
                            qT_sb[:, d * I + i0 : d * I + i0 + IB],
                            start=(d == 0),
                            stop=(d == EC - 1),
                        )
                    return sim

                def emit_pv(jc, expT):
                    for isub in range(NSUB):
                        lhs = expT[:, isub * P : (isub + 1) * P]
                        for eh in range(E // 512):
                            nc.tensor.matmul(
                                pv[isub][eh][:],
                                lhs,
                                v_sb[:, jc * E + eh * 512
                                     : jc * E + (eh + 1) * 512],
                                start=(jc == 0),
                                stop=(jc == NJC - 1),
                            )
                        nc.tensor.matmul(
                            den[isub][:],
                            lhs,
                            ones[:],
                            start=(jc == 0),
                            stop=(jc == NJC - 1),
                        )

                # pv/den for chunk jc are emitted after sim for chunk
                # jc+2, so the exp -> semaphore -> PE latency hides under
                # two full sim streams instead of poking a ~125ns bubble
                # into each cycle.
                pending = []
                for jc in range(NJC):
                    sim = emit_sim(jc)
                    expT = exp_pool.tile([P, IB], F16, tag="expT")
                    nc.scalar.activation(
                        expT[:], sim[:], mybir.ActivationFunctionType.Exp,
                        scale=SCALE,
                        bias=beta_sb[:, jc : jc + 1],
                    )
                    pending.append((jc, expT))
                    if len(pending) > 2:
                        emit_pv(*pending.pop(0))
                for item in pending:
                    emit_pv(*item)

                recip = small.tile([P, NSUB], F32, tag="recip")
                for isub in range(NSUB):
                    nc.vector.reciprocal(
                        recip[:, isub : isub + 1], den[isub][:]
                    )
                # All evictions on DVE (idle during attention) so the ACT
                # queue stays exp-only — an eviction burst on ACT at a block
                # boundary delays exp(0) and stalls the PE on the sim-bank
                # reuse semaphore.  Exception: the final block has no exp
                # work left, so its evictions split DVE/ACT to halve the
                # tail.  Both 512-wide halves land in one [P, E] tile so a
                # single DMA (~600ns issue each) covers a full row block.
                last = ib == I // IB - 1
                for isub in range(NSUB):
                    o = out_pool.tile([P, E], F16, tag="o")
                    for eh in range(E // 512):
                        dst = o[:, eh * 512 : (eh + 1) * 512]
                        if last and eh == 1:
                            nc.scalar.activation(
                                dst,
                                pv[isub][eh][:],
                                mybir.ActivationFunctionType.Copy,
                                scale=recip[:, isub : isub + 1],
                            )
                        else:
                            nc.vector.tensor_scalar_mul(
                                dst, pv[isub][eh][:], recip[:, isub : isub + 1]
                            )
                    nc.sync.dma_start(
                        out_d.ap()[i0 + isub * P : i0 + (isub + 1) * P, :],
                        o[:],
                    )

    nc.compile()
    return nc


def _get_nc():
    if "nc" not in _NC_CACHE:
        _NC_CACHE["nc"] = _build()
    return _NC_CACHE["nc"]


def kernel(q_in, k_v, Wq, bq, Wk, bk, Wv, bv):
    q_in = np.asarray(q_in, dtype=np.float32)
    k_v = np.asarray(k_v, dtype=np.float32)
    Wq = np.asarray(Wq, dtype=np.float32)
    Wk = np.asarray(Wk, dtype=np.float32)
    Wv = np.asarray(Wv, dtype=np.float32)
    bq = np.asarray(bq, dtype=np.float32)
    bv = np.asarray(bv, dtype=np.float32)

    nc = _get_nc()

    # sim = x_q (Wq Wk^T) x_k^T (+ bias terms, see module docstring).
    # proj_T consumes weights in [in, out] layout: W'[e, d] with
    # k'[d] = sum_e W'[e, d] x_k[e], and W' = (Wq Wk^T)^T = Wk Wq^T.
    M_t16 = np.ascontiguousarray((Wk @ Wq.T).astype(np.float16))
    Wv16 = np.ascontiguousarray(Wv.astype(np.float16))
    bv_bc = np.ascontiguousarray(np.broadcast_to(bv, (P, E)))
    u = Wk @ bq  # beta_j = SCALE * x_k[j] . u  (zero when bq == 0)

    in_maps = []
    for b in range(B):
        beta = (SCALE * (k_v[b] @ u)).astype(np.float32)
        beta_t = np.ascontiguousarray(beta.reshape(J // P, P).T)
        in_maps.append(
            {
                "q_inT": np.ascontiguousarray(q_in[b].T).astype(np.float16),
                "k_vT": np.ascontiguousarray(k_v[b].T).astype(np.float16),
                "M_t": M_t16,
                "Wv": Wv16,
                "beta_t": beta_t,
                "bv_bc": bv_bc,
            }
        )

    global LAST_RESULTS
    LAST_RESULTS = run_bass_kernel_spmd(
        nc, in_maps, core_ids=list(range(B)), **_RUN_KWARGS
    )
    return np.stack(
        [LAST_RESULTS.results[b]["out"].astype(np.float32) for b in range(B)]
    )


# revision 16
# speedup vs baseline: 1.0539x; 1.0161x over previous
"""Cross-attention Trainium2 Bass kernel.

Problem (per full input):
    q_in [8, 2048, 1024] f32, k_v [8, 2048, 1024] f32,
    Wq/Wk/Wv [1024, 1024] f32, bq/bk/bv [1024] f32
    q = q_in @ Wq + bq; k = k_v @ Wk + bk; v = k_v @ Wv + bv
    out = softmax(q k^T / sqrt(1024)) v        -> [8, 2048, 1024] f32

Sharding: data-parallel over batch, one batch per NeuronCore (8 cores).

Key algebraic reduction: q and k only ever appear through
    sim = (x_q Wq + bq)(x_k Wk + bk)^T
       = x_q (Wq Wk^T) x_k^T  +  [per-i shift, cancels in softmax]
         + (x_k Wk bq)_j      +  [const, cancels]
so with M := Wq Wk^T precomputed on the host (weight-only, O(E^3)) the
kernel needs just ONE projection k' = M x_k^T instead of separate q and
k projections — 2.15e9 of the 15e9 per-core MACs disappear.  The per-j
bias term beta_j = (x_k Wk bq)_j (zero for this problem's zero biases,
but handled generally) folds into the exp activation's per-partition
bias.  x_q feeds the attention matmul directly from HBM.

Per-core algorithm (I = J = 2048, E = D = 1024, P = 128):
  - Host pre-transposes activations to [E, I] and casts to fp16.
  - k'T[d,j] computed with the M chunk as the stationary operand (output
    comes out transposed, exactly the layout the attention matmul
    needs); v[j,e] computed with the x_kT chunk stationary.
  - Attention: simT[j,i] = k'T^T x_qT accumulated over d in PSUM; exp on
    the ACT engine with the 1/sqrt(E) scale and beta bias fused; PV
    accumulates sum_j expT[j,i] v[j,e] over all j in PSUM (unnormalized),
    the softmax denominator accumulates in parallel as an N=1 matmul
    against a ones vector (reusing the expT stationary); a per-partition
    reciprocal multiply normalizes at eviction.
  - exp is computed without max subtraction: sim ~ N(0,1) for this
    problem's distribution, so exp() stays comfortably inside fp16/fp32
    range and softmax is shift-invariant anyway.
  - Output is evicted and DMA'd as fp16 (rounding ~2.4e-4 relative, far
    under the 2e-2 gate); the host upcasts to fp32.
  - fp8 was evaluated and rejected: e4m3's 3 mantissa bits measure
    3e-2..6e-2 on the max-norm metric for any of sim/PV quantized
    (numpy study on the real data), over the 2e-2 gate.
"""

import numpy as np
from contextlib import ExitStack

import concourse.bass as bass
import concourse.mybir as mybir
import concourse.tile as tile
from concourse import bacc
from concourse.bass_utils import run_bass_kernel_spmd

B = 8
I = 2048  # query positions per batch
J = 2048  # key positions per batch
E = 1024  # embed dim
P = 128
EC = E // P  # 8 contraction chunks
SCALE = float(E) ** -0.5

F16 = mybir.dt.float16
F32 = mybir.dt.float32

# i-block size for the attention phase (sim moving free dim).  256 keeps the
# PSUM budget at 8 banks: 4 PV + up to 3 simT + 1 denominator.
IB = 256

# Module-level knobs test.py may override before the first kernel() call.
_RUN_KWARGS: dict = {}
LAST_RESULTS = None

_NC_CACHE: dict = {}


def _build():
    nc = bacc.Bacc("TRN2", target_bir_lowering=False, debug=False)

    q_inT = nc.dram_tensor("q_inT", [E, I], F16, kind="ExternalInput")
    k_vT = nc.dram_tensor("k_vT", [E, J], F16, kind="ExternalInput")
    M_t = nc.dram_tensor("M_t", [E, E], F16, kind="ExternalInput")
    Wv_d = nc.dram_tensor("Wv", [E, E], F16, kind="ExternalInput")
    # beta[p, jc]: SCALE * (x_k Wk bq)_j at j = jc*128 + p, fused into exp
    beta_d = nc.dram_tensor("beta_t", [P, J // P], F32, kind="ExternalInput")
    bv_bc = nc.dram_tensor("bv_bc", [P, E], F32, kind="ExternalInput")
    out_d = nc.dram_tensor("out", [I, E], F16, kind="ExternalOutput")

    with tile.TileContext(nc) as tc, ExitStack() as ctx:
        const = ctx.enter_context(tc.tile_pool(name="const", bufs=1))
        ones = const.tile([P, 1], F16)
        nc.vector.memset(ones[:], 1.0)
        beta_sb = const.tile([P, J // P], F32, tag="beta")
        bv_sb = const.tile([P, E], F32, tag="bv")

        # Persistent fp16 operands for the attention phase.
        # xqT/kT: chunk d lives at [:, d*I + i]  (layout [d, i] / [d, j])
        # v:     chunk jc lives at [:, jc*E + e] (layout [j, e])
        persist = ctx.enter_context(tc.tile_pool(name="persist", bufs=1))
        qT_sb = persist.tile([P, EC * I], F16, tag="qT")
        kT_sb = persist.tile([P, EC * J], F16, tag="kT")
        v_sb = persist.tile([P, (J // P) * E], F16, tag="v")

        # ---------------- phase A/B: projections ----------------
        with ExitStack() as ab:
            wpool = ab.enter_context(tc.tile_pool(name="wpool", bufs=1))
            # Both weight matrices in one tile: W w chunk e at
            # [:, w*E*EC + e*E + d]   ([128, 16384] f16 = 32KB/partition).
            # Chunk DMAs are emitted lazily, interleaved with the activation
            # chunk DMAs each phase consumes first, so the PE isn't stalled
            # at kernel start behind 4MB of weights it doesn't need yet.
            w_sb = wpool.tile([P, 2 * EC * E], F16, tag="W")
            w_off = {"M": 0, "Wv": EC * E}
            w_dram = {"M": M_t, "Wv": Wv_d}

            def load_w_chunk(w, e, eng=None):
                # M rides the scalar engine's HWDGE queue, Wv the gpsimd SWDGE
                # queue, so both stream in parallel with the activation chunks
                # on the sync queue.  All weight issues happen in the first
                # ~10us, while those engines have no other work yet —
                # DMA_DIRECT2D costs ~600ns of issue time on the queueing
                # engine, so it must never sit in front of exp/eviction work.
                (eng or nc.scalar).dma_start(
                    w_sb[:, w_off[w] + e * E : w_off[w] + (e + 1) * E],
                    w_dram[w].ap()[e * P : (e + 1) * P, :],
                )

            xpool = ab.enter_context(tc.tile_pool(name="xpool", bufs=2))
            ppool = ab.enter_context(
                tc.tile_pool(name="proj_ps", bufs=4, space="PSUM")
            )

            H = 1024  # half of the j range handled per streamed xT tile

            def load_half(src, h, with_w=None):
                xh = xpool.tile([P, EC * H], F16, tag="xT")
                for e in range(EC):
                    if with_w is not None:
                        load_w_chunk(with_w, e)
                    nc.sync.dma_start(
                        xh[:, e * H : (e + 1) * H],
                        src.ap()[e * P : (e + 1) * P, h * H : (h + 1) * H],
                    )
                return xh

            def load_qT(ec_range):
                # x_q needs no projection: DMA it straight into the
                # attention-phase operand slot.  Emitted on the sync queue
                # AFTER both x_k halves so queue FIFO order prioritizes the
                # projection-critical bytes (HBM bandwidth is shared across
                # queues; order is the only priority control).
                for e in range(*ec_range):
                    nc.sync.dma_start(
                        qT_sb[:, e * I : (e + 1) * I],
                        q_inT.ap()[e * P : (e + 1) * P, :],
                    )

            def proj_T(xh, h, wname, dst):
                # dst[d, n] = sum_e W[e,d] x[n,e], n in this half
                for d in range(EC):
                    for ib in range(H // 512):
                        ps = ppool.tile([P, 512], F32, tag="proj")
                        for e in range(EC):
                            nc.tensor.matmul(
                                ps[:],
                                w_sb[:, w_off[wname] + e * E + d * P
                                     : w_off[wname] + e * E + (d + 1) * P],
                                xh[:, e * H + ib * 512 : e * H + (ib + 1) * 512],
                                start=(e == 0),
                                stop=(e == EC - 1),
                            )
                        nc.scalar.activation(
                            dst[:, d * I + h * H + ib * 512
                                : d * I + h * H + (ib + 1) * 512],
                            ps[:],
                            mybir.ActivationFunctionType.Identity,
                        )

            def proj_v(xh, h):
                # v[j, e] = sum_e' k_v[j, e'] Wv[e', e] + bv[e], j in this half
                for jc in range(H // P):
                    jg = h * (H // P) + jc
                    for eh in range(E // 512):
                        ps = ppool.tile([P, 512], F32, tag="proj")
                        for e in range(EC):
                            nc.tensor.matmul(
                                ps[:],
                                xh[:, e * H + jc * P : e * H + (jc + 1) * P],
                                w_sb[:, w_off["Wv"] + e * E + eh * 512
                                     : w_off["Wv"] + e * E + (eh + 1) * 512],
                                start=(e == 0),
                                stop=(e == EC - 1),
                            )
                        nc.vector.tensor_add(
                            v_sb[:, jg * E + eh * 512 : jg * E + (eh + 1) * 512],
                            ps[:],
                            bv_sb[:, eh * 512 : (eh + 1) * 512],
                        )

            # Warmup spin: ~14 throwaway matmuls on a zeroed tile keep the PE
            # busy from ~0.5us so the HAM activity window fills and the clock
            # gate opens (1.2 -> 2.4 GHz) BEFORE the first real operands land
            # (~6us).  Without this the whole first projection half runs at
            # half clock (observed: K=4/8 until 36us).
            warm_ps_pool = ab.enter_context(
                tc.tile_pool(name="warm_ps", bufs=1, space="PSUM")
            )
            warm_sb = const.tile([P, 512], F16, tag="warm")
            nc.vector.memset(warm_sb[:], 0.0)
            warm_ps = warm_ps_pool.tile([P, 512], F32, tag="warm")
            for _ in range(14):
                nc.tensor.matmul(
                    warm_ps[:], warm_sb[:, 0:P], warm_sb[:],
                    start=True, stop=True, skip_group_check=True,
                )

            nc.gpsimd.dma_start(beta_sb[:], beta_d.ap())
            nc.gpsimd.dma_start(bv_sb[:], bv_bc.ap())
            xh0 = load_half(k_vT, 0, with_w="M")
            for e in range(EC):
                load_w_chunk("Wv", e)
            xh1 = load_half(k_vT, 1)
            load_qT((0, EC))
            proj_T(xh0, 0, "M", kT_sb)
            proj_v(xh0, 0)
            proj_T(xh1, 1, "M", kT_sb)
            proj_v(xh1, 1)

        # ---------------- phase C: attention ----------------
        with ExitStack() as c:
            sim_ps_pool = c.enter_context(
                tc.tile_pool(name="sim_ps", bufs=2, space="PSUM")
            )
            pv_ps_pool = c.enter_context(
                tc.tile_pool(name="pv_ps", bufs=4, space="PSUM")
            )
            # NOTE: matmul start=True clears has_written for the WHOLE PSUM
            # bank, so each accumulation group needs its own bank — one den
            # tile per i-subtile, never two groups in one tile.
            den_ps_pool = c.enter_context(
                tc.tile_pool(name="den_ps", bufs=2, space="PSUM")
            )
            exp_pool = c.enter_context(tc.tile_pool(name="exp", bufs=4))
            out_pool = c.enter_context(tc.tile_pool(name="outsb", bufs=6))
            small = c.enter_context(tc.tile_pool(name="small", bufs=2))

            NSUB = IB // P  # i-subtiles per block
            NJC = J // P

            for ib in range(I // IB):
                i0 = ib * IB
                pv = [
                    [
                        pv_ps_pool.tile(
                            [P, 512], F32, tag="pv", name=f"pv_{ib}_{s}_{eh}"
                        )
                        for eh in range(E // 512)
                    ]
                    for s in range(NSUB)
                ]
                den = [
                    den_ps_pool.tile([P, 1], F32, tag="den", name=f"den_{ib}_{s}")
                    for s in range(NSUB)
                ]

                def emit_sim(jc):
                    sim = sim_ps_pool.tile([P, IB], F32, tag="sim",
                                           name=f"sim_{ib}_{jc}")
                    for d in range(EC):
                        nc.tensor.matmul(
                            sim[:],
                            kT_sb[:, d * J + jc * P : d * J + (jc + 1) * P],
                            qT_sb[:, d * I + i0 : d * I + i0 + IB],
                            start=(d == 0),
                            stop=(d == EC - 1),
                        )
                    return sim

                def emit_pv(jc, expT):
                    for isub in range(NSUB):
                        lhs = expT[:, isub * P : (isub + 1) * P]
                        for eh in range(E // 512):
                            nc.tensor.matmul(
                                pv[isub][eh][:],
                                lhs,
                                v_sb[:, jc * E + eh * 512
                                     : jc * E + (eh + 1) * 512],
                                start=(jc == 0),
                                stop=(jc == NJC - 1),
                            )
                        nc.tensor.matmul(
                            den[isub][:],
                            lhs,
                            ones[:],
                            start=(jc == 0),
                            stop=(jc == NJC - 1),
                        )

                # pv/den for chunk jc are emitted after sim for chunk
                # jc+2, so the exp -> semaphore -> PE latency hides under
                # two full sim streams instead of poking a ~125ns bubble
                # into each cycle.
                pending = []
                for jc in range(NJC):
                    sim = emit_sim(jc)
                    expT = exp_pool.tile([P, IB], F16, tag="expT")
                    nc.scalar.activation(
                        expT[:], sim[:], mybir.ActivationFunctionType.Exp,
                        scale=SCALE,
                        bias=beta_sb[:, jc : jc + 1],
                    )
                    pending.append((jc, expT))
                    if len(pending) > 2:
                        emit_pv(*pending.pop(0))
                for item in pending:
                    emit_pv(*item)

                recip = small.tile([P, NSUB], F32, tag="recip")
                for isub in range(NSUB):
                    nc.vector.reciprocal(
                        recip[:, isub : isub + 1], den[isub][:]
                    )
                # All evictions on DVE (idle during attention) so the ACT
                # queue stays exp-only — an eviction burst on ACT at a block
                # boundary delays exp(0) and stalls the PE on the sim-bank
                # reuse semaphore.  Exception: the final block has no exp
                # work left, so its evictions split DVE/ACT to halve the
                # tail.  Both 512-wide halves land in one [P, E] tile so a
                # single DMA (~600ns issue each) covers a full row block.
                last = ib == I // IB - 1
                for isub in range(NSUB):
                    o = out_pool.tile([P, E], F16, tag="o")
                    for eh in range(E // 512):
                        dst = o[:, eh * 512 : (eh + 1) * 512]
                        if last and eh == 1:
                            nc.scalar.activation(
                                dst,
                                pv[isub][eh][:],
                                mybir.ActivationFunctionType.Copy,
                                scale=recip[:, isub : isub + 1],
                            )
                        else:
                            nc.vector.tensor_scalar_mul(
                                dst, pv[isub][eh][:], recip[:, isub : isub + 1]
                            )
                    nc.sync.dma_start(
                        out_d.ap()[i0 + isub * P : i0 + (isub + 1) * P, :],
                        o[:],
                    )

    nc.compile()
    return nc


def _get_nc():
    if "nc" not in _NC_CACHE:
        _NC_CACHE["nc"] = _build()
    return _NC_CACHE["nc"]


def kernel(q_in, k_v, Wq, bq, Wk, bk, Wv, bv):
    q_in = np.asarray(q_in, dtype=np.float32)
    k_v = np.asarray(k_v, dtype=np.float32)
    Wq = np.asarray(Wq, dtype=np.float32)
    Wk = np.asarray(Wk, dtype=np.float32)
    Wv = np.asarray(Wv, dtype=np.float32)
    bq = np.asarray(bq, dtype=np.float32)
    bv = np.asarray(bv, dtype=np.float32)

    nc = _get_nc()

    # sim = x_q (Wq Wk^T) x_k^T (+ bias terms, see module docstring).
    # proj_T consumes weights in [in, out] layout: W'[e, d] with
    # k'[d] = sum_e W'[e, d] x_k[e], and W' = (Wq Wk^T)^T = Wk Wq^T.
    M_t16 = np.ascontiguousarray((Wk @ Wq.T).astype(np.float16))
    Wv16 = np.ascontiguousarray(Wv.astype(np.float16))
    bv_bc = np.ascontiguousarray(np.broadcast_to(bv, (P, E)))
    u = Wk @ bq  # beta_j = SCALE * x_k[j] . u  (zero when bq == 0)

    in_maps = []
    for b in range(B):
        beta = (SCALE * (k_v[b] @ u)).astype(np.float32)
        beta_t = np.ascontiguousarray(beta.reshape(J // P, P).T)
        in_maps.append(
            {
                "q_inT": np.ascontiguousarray(q_in[b].T).astype(np.float16),
                "k_vT": np.ascontiguousarray(k_v[b].T).astype(np.float16),
                "M_t": M_t16,
                "Wv": Wv16,
                "beta_t": beta_t,
                "bv_bc": bv_bc,
            }
        )

    global LAST_RESULTS
    LAST_RESULTS = run_bass_kernel_spmd(
        nc, in_maps, core_ids=list(range(B)), **_RUN_KWARGS
    )
    return np.stack(
        [LAST_RESULTS.results[b]["out"].astype(np.float32) for b in range(B)]
    )


# revision 17
# speedup vs baseline: 1.0712x; 1.0164x over previous
"""Cross-attention Trainium2 Bass kernel.

Problem (per full input):
    q_in [8, 2048, 1024] f32, k_v [8, 2048, 1024] f32,
    Wq/Wk/Wv [1024, 1024] f32, bq/bk/bv [1024] f32
    q = q_in @ Wq + bq; k = k_v @ Wk + bk; v = k_v @ Wv + bv
    out = softmax(q k^T / sqrt(1024)) v        -> [8, 2048, 1024] f32

Sharding: data-parallel over batch, one batch per NeuronCore (8 cores).

Key algebraic reduction: q and k only ever appear through
    sim = (x_q Wq + bq)(x_k Wk + bk)^T
       = x_q (Wq Wk^T) x_k^T  +  [per-i shift, cancels in softmax]
         + (x_k Wk bq)_j      +  [const, cancels]
so with M := Wq Wk^T precomputed on the host (weight-only, O(E^3)) the
kernel needs just ONE projection k' = M x_k^T instead of separate q and
k projections — 2.15e9 of the 15e9 per-core MACs disappear.  The per-j
bias term beta_j = (x_k Wk bq)_j (zero for this problem's zero biases,
but handled generally) folds into the exp activation's per-partition
bias.  x_q feeds the attention matmul directly from HBM.

Per-core algorithm (I = J = 2048, E = D = 1024, P = 128):
  - Host pre-transposes activations to [E, I] and casts to fp16.
  - k'T[d,j] computed with the M chunk as the stationary operand (output
    comes out transposed, exactly the layout the attention matmul
    needs); v[j,e] computed with the x_kT chunk stationary.
  - Attention: simT[j,i] = k'T^T x_qT accumulated over d in PSUM; exp on
    the ACT engine with the 1/sqrt(E) scale and beta bias fused; PV
    accumulates sum_j expT[j,i] v[j,e] over all j in PSUM (unnormalized),
    the softmax denominator accumulates in parallel as an N=1 matmul
    against a ones vector (reusing the expT stationary); a per-partition
    reciprocal multiply normalizes at eviction.
  - exp is computed without max subtraction: sim ~ N(0,1) for this
    problem's distribution, so exp() stays comfortably inside fp16/fp32
    range and softmax is shift-invariant anyway.
  - Output is evicted and DMA'd as fp16 (rounding ~2.4e-4 relative, far
    under the 2e-2 gate); the host upcasts to fp32.
  - fp8 was evaluated and rejected: e4m3's 3 mantissa bits measure
    3e-2..6e-2 on the max-norm metric for any of sim/PV quantized
    (numpy study on the real data), over the 2e-2 gate.
"""

import numpy as np
from contextlib import ExitStack

import concourse.bass as bass
import concourse.mybir as mybir
import concourse.tile as tile
from concourse import bacc
from concourse.bass_utils import run_bass_kernel_spmd

B = 8
I = 2048  # query positions per batch
J = 2048  # key positions per batch
E = 1024  # embed dim
P = 128
EC = E // P  # 8 contraction chunks
SCALE = float(E) ** -0.5

F16 = mybir.dt.float16
F32 = mybir.dt.float32

# i-block size for the attention phase (sim moving free dim).  256 keeps the
# PSUM budget at 8 banks: 4 PV + up to 3 simT + 1 denominator.
IB = 256

# Module-level knobs test.py may override before the first kernel() call.
_RUN_KWARGS: dict = {}
LAST_RESULTS = None

_NC_CACHE: dict = {}


def _build():
    nc = bacc.Bacc("TRN2", target_bir_lowering=False, debug=False)

    q_inT = nc.dram_tensor("q_inT", [E, I], F16, kind="ExternalInput")
    k_vT = nc.dram_tensor("k_vT", [E, J], F16, kind="ExternalInput")
    M_t = nc.dram_tensor("M_t", [E, E], F16, kind="ExternalInput")
    Wv_d = nc.dram_tensor("Wv", [E, E], F16, kind="ExternalInput")
    # beta[p, jc]: SCALE * (x_k Wk bq)_j at j = jc*128 + p, fused into exp
    beta_d = nc.dram_tensor("beta_t", [P, J // P], F32, kind="ExternalInput")
    bv_bc = nc.dram_tensor("bv_bc", [P, E], F32, kind="ExternalInput")
    out_d = nc.dram_tensor("out", [I, E], F16, kind="ExternalOutput")

    with tile.TileContext(nc) as tc, ExitStack() as ctx:
        const = ctx.enter_context(tc.tile_pool(name="const", bufs=1))
        ones = const.tile([P, 1], F16)
        nc.vector.memset(ones[:], 1.0)
        beta_sb = const.tile([P, J // P], F32, tag="beta")
        bv_sb = const.tile([P, E], F32, tag="bv")

        # Persistent fp16 operands for the attention phase.
        # xqT/kT: chunk d lives at [:, d*I + i]  (layout [d, i] / [d, j])
        # v:     chunk jc lives at [:, jc*E + e] (layout [j, e])
        persist = ctx.enter_context(tc.tile_pool(name="persist", bufs=1))
        qT_sb = persist.tile([P, EC * I], F16, tag="qT")
        kT_sb = persist.tile([P, EC * J], F16, tag="kT")
        v_sb = persist.tile([P, (J // P) * E], F16, tag="v")

        # ---------------- phase A/B: projections ----------------
        with ExitStack() as ab:
            wpool = ab.enter_context(tc.tile_pool(name="wpool", bufs=1))
            # Both weight matrices in one tile: W w chunk e at
            # [:, w*E*EC + e*E + d]   ([128, 16384] f16 = 32KB/partition).
            # Chunk DMAs are emitted lazily, interleaved with the activation
            # chunk DMAs each phase consumes first, so the PE isn't stalled
            # at kernel start behind 4MB of weights it doesn't need yet.
            w_sb = wpool.tile([P, 2 * EC * E], F16, tag="W")
            w_off = {"M": 0, "Wv": EC * E}
            w_dram = {"M": M_t, "Wv": Wv_d}

            def load_w_chunk(w, e, eng=None):
                # M rides the scalar engine's HWDGE queue, Wv the gpsimd SWDGE
                # queue, so both stream in parallel with the activation chunks
                # on the sync queue.  All weight issues happen in the first
                # ~10us, while those engines have no other work yet —
                # DMA_DIRECT2D costs ~600ns of issue time on the queueing
                # engine, so it must never sit in front of exp/eviction work.
                (eng or nc.scalar).dma_start(
                    w_sb[:, w_off[w] + e * E : w_off[w] + (e + 1) * E],
                    w_dram[w].ap()[e * P : (e + 1) * P, :],
                )

            xpool = ab.enter_context(tc.tile_pool(name="xpool", bufs=2))
            ppool = ab.enter_context(
                tc.tile_pool(name="proj_ps", bufs=4, space="PSUM")
            )

            H = 1024  # half of the j range handled per streamed xT tile

            def load_half(src, h, with_w=None):
                xh = xpool.tile([P, EC * H], F16, tag="xT")
                for e in range(EC):
                    if with_w is not None:
                        load_w_chunk(with_w, e)
                    nc.sync.dma_start(
                        xh[:, e * H : (e + 1) * H],
                        src.ap()[e * P : (e + 1) * P, h * H : (h + 1) * H],
                    )
                return xh

            def load_qT(ec_range):
                # x_q needs no projection: DMA it straight into the
                # attention-phase operand slot.  Emitted on the sync queue
                # AFTER both x_k halves so queue FIFO order prioritizes the
                # projection-critical bytes (HBM bandwidth is shared across
                # queues; order is the only priority control).
                for e in range(*ec_range):
                    nc.sync.dma_start(
                        qT_sb[:, e * I : (e + 1) * I],
                        q_inT.ap()[e * P : (e + 1) * P, :],
                    )

            def proj_T(xh, h, wname, dst):
                # dst[d, n] = sum_e W[e,d] x[n,e], n in this half
                for d in range(EC):
                    for ib in range(H // 512):
                        ps = ppool.tile([P, 512], F32, tag="proj")
                        for e in range(EC):
                            nc.tensor.matmul(
                                ps[:],
                                w_sb[:, w_off[wname] + e * E + d * P
                                     : w_off[wname] + e * E + (d + 1) * P],
                                xh[:, e * H + ib * 512 : e * H + (ib + 1) * 512],
                                start=(e == 0),
                                stop=(e == EC - 1),
                            )
                        nc.scalar.activation(
                            dst[:, d * I + h * H + ib * 512
                                : d * I + h * H + (ib + 1) * 512],
                            ps[:],
                            mybir.ActivationFunctionType.Identity,
                        )

            def proj_v(xh, h):
                # v[j, e] = sum_e' k_v[j, e'] Wv[e', e] + bv[e], j in this half
                for jc in range(H // P):
                    jg = h * (H // P) + jc
                    for eh in range(E // 512):
                        ps = ppool.tile([P, 512], F32, tag="proj")
                        for e in range(EC):
                            nc.tensor.matmul(
                                ps[:],
                                xh[:, e * H + jc * P : e * H + (jc + 1) * P],
                                w_sb[:, w_off["Wv"] + e * E + eh * 512
                                     : w_off["Wv"] + e * E + (eh + 1) * 512],
                                start=(e == 0),
                                stop=(e == EC - 1),
                            )
                        nc.vector.tensor_add(
                            v_sb[:, jg * E + eh * 512 : jg * E + (eh + 1) * 512],
                            ps[:],
                            bv_sb[:, eh * 512 : (eh + 1) * 512],
                        )

            # DMA priority (HBM bandwidth is shared across queues, FIFO within
            # one): the critical first set — x-h0 on sync + M on scalar — owns
            # both queues from t=0.  Everything else follows in deadline
            # order.  proj_T(h1) runs BEFORE proj_v(h0) so the Wv deadline
            # moves from ~35us to ~62us, letting Wv queue behind x-h1.
            nc.gpsimd.dma_start(beta_sb[:], beta_d.ap())
            nc.gpsimd.dma_start(bv_sb[:], bv_bc.ap())
            xh0 = load_half(k_vT, 0, with_w="M")
            xh1 = load_half(k_vT, 1)
            for e in range(EC):
                load_w_chunk("Wv", e, eng=nc.sync)
            load_qT((0, EC))
            proj_T(xh0, 0, "M", kT_sb)
            proj_T(xh1, 1, "M", kT_sb)
            proj_v(xh0, 0)
            proj_v(xh1, 1)

        # ---------------- phase C: attention ----------------
        with ExitStack() as c:
            sim_ps_pool = c.enter_context(
                tc.tile_pool(name="sim_ps", bufs=2, space="PSUM")
            )
            pv_ps_pool = c.enter_context(
                tc.tile_pool(name="pv_ps", bufs=4, space="PSUM")
            )
            # NOTE: matmul start=True clears has_written for the WHOLE PSUM
            # bank, so each accumulation group needs its own bank — one den
            # tile per i-subtile, never two groups in one tile.
            den_ps_pool = c.enter_context(
                tc.tile_pool(name="den_ps", bufs=2, space="PSUM")
            )
            exp_pool = c.enter_context(tc.tile_pool(name="exp", bufs=4))
            out_pool = c.enter_context(tc.tile_pool(name="outsb", bufs=6))
            small = c.enter_context(tc.tile_pool(name="small", bufs=2))

            NSUB = IB // P  # i-subtiles per block
            NJC = J // P

            for ib in range(I // IB):
                i0 = ib * IB
                pv = [
                    [
                        pv_ps_pool.tile(
                            [P, 512], F32, tag="pv", name=f"pv_{ib}_{s}_{eh}"
                        )
                        for eh in range(E // 512)
                    ]
                    for s in range(NSUB)
                ]
                den = [
                    den_ps_pool.tile([P, 1], F32, tag="den", name=f"den_{ib}_{s}")
                    for s in range(NSUB)
                ]

                def emit_sim(jc):
                    sim = sim_ps_pool.tile([P, IB], F32, tag="sim",
                                           name=f"sim_{ib}_{jc}")
                    for d in range(EC):
                        nc.tensor.matmul(
                            sim[:],
                            kT_sb[:, d * J + jc * P : d * J + (jc + 1) * P],
                            qT_sb[:, d * I + i0 : d * I + i0 + IB],
                            start=(d == 0),
                            stop=(d == EC - 1),
                        )
                    return sim

                def emit_pv(jc, expT):
                    for isub in range(NSUB):
                        lhs = expT[:, isub * P : (isub + 1) * P]
                        for eh in range(E // 512):
                            nc.tensor.matmul(
                                pv[isub][eh][:],
                                lhs,
                                v_sb[:, jc * E + eh * 512
                                     : jc * E + (eh + 1) * 512],
                                start=(jc == 0),
                                stop=(jc == NJC - 1),
                            )
                        nc.tensor.matmul(
                            den[isub][:],
                            lhs,
                            ones[:],
                            start=(jc == 0),
                            stop=(jc == NJC - 1),
                        )

                # pv/den for chunk jc are emitted after sim for chunk
                # jc+2, so the exp -> semaphore -> PE latency hides under
                # two full sim streams instead of poking a ~125ns bubble
                # into each cycle.
                pending = []
                for jc in range(NJC):
                    sim = emit_sim(jc)
                    expT = exp_pool.tile([P, IB], F16, tag="expT")
                    nc.scalar.activation(
                        expT[:], sim[:], mybir.ActivationFunctionType.Exp,
                        scale=SCALE,
                        bias=beta_sb[:, jc : jc + 1],
                    )
                    pending.append((jc, expT))
                    if len(pending) > 2:
                        emit_pv(*pending.pop(0))
                for item in pending:
                    emit_pv(*item)

                recip = small.tile([P, NSUB], F32, tag="recip")
                for isub in range(NSUB):
                    nc.vector.reciprocal(
                        recip[:, isub : isub + 1], den[isub][:]
                    )
                # All evictions on DVE (idle during attention) so the ACT
                # queue stays exp-only — an eviction burst on ACT at a block
                # boundary delays exp(0) and stalls the PE on the sim-bank
                # reuse semaphore.  Exception: the final block has no exp
                # work left, so its evictions split DVE/ACT to halve the
                # tail.  Both 512-wide halves land in one [P, E] tile so a
                # single DMA (~600ns issue each) covers a full row block.
                last = ib == I // IB - 1
                for isub in range(NSUB):
                    o = out_pool.tile([P, E], F16, tag="o")
                    for eh in range(E // 512):
                        dst = o[:, eh * 512 : (eh + 1) * 512]
                        if last and eh == 1:
                            nc.scalar.activation(
                                dst,
                                pv[isub][eh][:],
                                mybir.ActivationFunctionType.Copy,
                                scale=recip[:, isub : isub + 1],
                            )
                        else:
                            nc.vector.tensor_scalar_mul(
                                dst, pv[isub][eh][:], recip[:, isub : isub + 1]
                            )
                    nc.sync.dma_start(
                        out_d.ap()[i0 + isub * P : i0 + (isub + 1) * P, :],
                        o[:],
                    )

    nc.compile()
    return nc


def _get_nc():
    if "nc" not in _NC_CACHE:
        _NC_CACHE["nc"] = _build()
    return _NC_CACHE["nc"]


def kernel(q_in, k_v, Wq, bq, Wk, bk, Wv, bv):
    q_in = np.asarray(q_in, dtype=np.float32)
    k_v = np.asarray(k_v, dtype=np.float32)
    Wq = np.asarray(Wq, dtype=np.float32)
    Wk = np.asarray(Wk, dtype=np.float32)
    Wv = np.asarray(Wv, dtype=np.float32)
    bq = np.asarray(bq, dtype=np.float32)
    bv = np.asarray(bv, dtype=np.float32)

    nc = _get_nc()

    # sim = x_q (Wq Wk^T) x_k^T (+ bias terms, see module docstring).
    # proj_T consumes weights in [in, out] layout: W'[e, d] with
    # k'[d] = sum_e W'[e, d] x_k[e], and W' = (Wq Wk^T)^T = Wk Wq^T.
    M_t16 = np.ascontiguousarray((Wk @ Wq.T).astype(np.float16))
    Wv16 = np.ascontiguousarray(Wv.astype(np.float16))
    bv_bc = np.ascontiguousarray(np.broadcast_to(bv, (P, E)))
    u = Wk @ bq  # beta_j = SCALE * x_k[j] . u  (zero when bq == 0)

    in_maps = []
    for b in range(B):
        beta = (SCALE * (k_v[b] @ u)).astype(np.float32)
        beta_t = np.ascontiguousarray(beta.reshape(J // P, P).T)
        in_maps.append(
            {
                "q_inT": np.ascontiguousarray(q_in[b].T).astype(np.float16),
                "k_vT": np.ascontiguousarray(k_v[b].T).astype(np.float16),
                "M_t": M_t16,
                "Wv": Wv16,
                "beta_t": beta_t,
                "bv_bc": bv_bc,
            }
        )

    global LAST_RESULTS
    LAST_RESULTS = run_bass_kernel_spmd(
        nc, in_maps, core_ids=list(range(B)), **_RUN_KWARGS
    )
    return np.stack(
        [LAST_RESULTS.results[b]["out"].astype(np.float32) for b in range(B)]
    )
